# revision 43
# baseline (speedup 1.0000x reference)
"""BiMamba block kernel for 8 TRN2 NeuronCores.

Sharding: cores 0-3 run the fwd Mamba pass, cores 4-7 the bwd pass (on
time-reversed x). Within each 4-core group, d_inner (2048) is sharded
into 4 slices of 512 channels. out_proj and the fusion matmul are folded
into one [512, 1024] weight per core; partial outputs are summed with a
per-group ReduceScatter and reassembled on host.

Layout on device is feature-major [d, t]: channels in partitions, time in
the free dimension, so the selective scan maps onto tensor_tensor_scan
(one recurrence per partition lane, scanned along free/time).

v2 vs baseline (1913 us):
- depthwise conv as 4 diag-matmuls on the (idle) PE instead of STT ops
  on the bottleneck DVE; silu applied straight from PSUM.
- delta stored bf16 (2x faster dA exps on ScalarE); softplus batched as
  all-Exp-then-all-Ln (2 act-table loads instead of 32; each table
  switch costs ~10us of ACT_TABLE_LOADs).
- x_proj AllReduce in bf16; B/C rows broadcast straight out of the
  reduced dram tensor (no staging roundtrip).
- delta double-buffered so b1's softplus can run during scan(0)
  (baseline stalled ~130us on this WAR hazard).
- scan-loop emission software-pipelined: dA/dBu produced 2 iterations
  ahead; b1 prep emitted in small units interleaved into scan(0)'s
  emission; outmm(0)/zrec(1)/RS(0) interleaved into scan(1)'s.
- acc adds on GpSimd; every gp_mod-th dBu mult offloaded there too;
  scan output h written in-place over dBu, gated p in-place over h;
  yg written into the dead xs tile (no extra SBUF).
- acc in bf16 (SBUF pressure); acc init (D*u) via ScalarE scale-copy.
"""

import os
import sys

import numpy as np

sys.path.insert(0, "/opt/trn_rl_repo")

B = 2
L = 2048
DM = 1024
DI = 2048
DS = 512          # d_inner shard per core
N = 16            # d_state
R = 64            # dt_rank
NB = DS // 128    # 4 channel blocks of 128 per core
K_CONV = 4
NI = N * NB       # scan iterations per batch (n-major, j-minor)

_CACHE = {}


def build_program(data_dtype="bfloat16", scan_dtype="bfloat16",
                  acc_dtype="bfloat16", gp_mod=3, scan_inplace=True):
    from concourse import bacc, mybir, tile

    F32 = mybir.dt.float32
    DDT = getattr(mybir.dt, data_dtype)   # matmul inputs / data tensors
    SDT = getattr(mybir.dt, scan_dtype)   # scan-block tensors
    ADT = getattr(mybir.dt, acc_dtype)    # state accumulator
    ALU = mybir.AluOpType
    ACT = mybir.ActivationFunctionType

    nc = bacc.Bacc(
        "TRN2", target_bir_lowering=False, debug=False, num_devices=8
    )

    # ---- external inputs (per-core, host-prepped) ----
    xT_d = nc.dram_tensor("xT", [B, DM, L], DDT, kind="ExternalInput")
    w_xs_d = nc.dram_tensor("w_xs", [DM, DS], DDT, kind="ExternalInput")
    w_z_d = nc.dram_tensor("w_z", [DM, DS], DDT, kind="ExternalInput")
    w_xp_d = nc.dram_tensor("w_xp", [DS, 96], DDT, kind="ExternalInput")
    w_dt_d = nc.dram_tensor("w_dt", [R, DS], DDT, kind="ExternalInput")
    w_out_d = nc.dram_tensor("w_out", [DS, DM], DDT, kind="ExternalInput")
    conv_diag_d = nc.dram_tensor(
        "conv_diag", [128, NB * K_CONV * 128], DDT, kind="ExternalInput"
    )
    conv_b_d = nc.dram_tensor("conv_b", [128, NB], F32, kind="ExternalInput")
    dtb_d = nc.dram_tensor("dtb", [128, NB], F32, kind="ExternalInput")
    dskip_d = nc.dram_tensor("dskip", [128, NB], F32, kind="ExternalInput")
    a_pack_d = nc.dram_tensor("a_pack", [128, NB * N], F32, kind="ExternalInput")

    out_d = nc.dram_tensor("out", [B * L // 4, DM], DDT, kind="ExternalOutput")

    # ---- internal dram (split per batch: whole-tensor dependency
    # tracking would otherwise serialize b1's AllReduce behind all of
    # scan(0)'s B/C broadcast reads) ----
    xdbl_loc = [nc.dram_tensor(f"xdbl_loc{b}", [96, L], DDT) for b in range(B)]
    xdbl_red = [nc.dram_tensor(f"xdbl_red{b}", [96, L], DDT) for b in range(B)]
    part_out = [nc.dram_tensor(f"part_out{b}", [L, DM], DDT) for b in range(B)]
    rs_out = nc.dram_tensor("rs_out", [B * L // 4, DM], DDT)

    GROUPS = [[0, 1, 2, 3], [4, 5, 6, 7]]
    LP = L + 3                     # xs block stride (3 left-pad cols)

    with tile.TileContext(nc) as tc:
        with (
            tc.tile_pool(name="const", bufs=1) as cpool,
            tc.tile_pool(name="resid", bufs=1) as rpool,
            tc.tile_pool(name="dresid", bufs=2) as dpool,
            tc.tile_pool(name="work", bufs=2) as wpool,
            tc.tile_pool(name="scan3", bufs=3) as s3pool,
            tc.tile_pool(name="psum", bufs=2, space="PSUM") as ppool,
            tc.tile_pool(name="psum_o", bufs=2, space="PSUM") as opool,
        ):
            # ---- load weights/constants once ----
            w_xs_sb = cpool.tile([128, 8 * DS], DDT, tag="wxs")
            w_z_sb = cpool.tile([128, 8 * DS], DDT, tag="wz")
            for mt in range(8):
                nc.sync.dma_start(
                    out=w_xs_sb[:, mt * DS:(mt + 1) * DS],
                    in_=w_xs_d.ap()[mt * 128:(mt + 1) * 128, :],
                )
                nc.sync.dma_start(
                    out=w_z_sb[:, mt * DS:(mt + 1) * DS],
                    in_=w_z_d.ap()[mt * 128:(mt + 1) * 128, :],
                )
            w_xp_sb = cpool.tile([128, NB * 96], DDT, tag="wxp")
            for j in range(NB):
                nc.sync.dma_start(
                    out=w_xp_sb[:, j * 96:(j + 1) * 96],
                    in_=w_xp_d.ap()[j * 128:(j + 1) * 128, :],
                )
            w_dt_sb = cpool.tile([R, DS], DDT, tag="wdt")
            nc.sync.dma_start(out=w_dt_sb[:, :], in_=w_dt_d.ap()[:, :])
            w_out_sb = cpool.tile([128, NB * DM], DDT, tag="wout")
            for j in range(NB):
                nc.sync.dma_start(
                    out=w_out_sb[:, j * DM:(j + 1) * DM],
                    in_=w_out_d.ap()[j * 128:(j + 1) * 128, :],
                )
            conv_diag_sb = cpool.tile([128, NB * K_CONV * 128], DDT, tag="convd")
            nc.sync.dma_start(out=conv_diag_sb[:, :], in_=conv_diag_d.ap()[:, :])
            conv_b_sb = cpool.tile([128, NB], F32, tag="convb")
            nc.sync.dma_start(out=conv_b_sb[:, :], in_=conv_b_d.ap()[:, :])
            dtb_sb = cpool.tile([128, NB], F32, tag="dtb")
            nc.sync.dma_start(out=dtb_sb[:, :], in_=dtb_d.ap()[:, :])
            dskip_sb = cpool.tile([128, NB], F32, tag="dskip")
            nc.sync.dma_start(out=dskip_sb[:, :], in_=dskip_d.ap()[:, :])
            a_sb = cpool.tile([128, NB * N], F32, tag="apack")
            nc.sync.dma_start(out=a_sb[:, :], in_=a_pack_d.ap()[:, :])

            st = [{} for _ in range(B)]

            # ---------------- phase helpers (emit "units") ----------------

            def mm_unit(b, tch):
                """in-proj xs matmuls for one 512-col time chunk."""
                s = st[b]
                if tch == 0:
                    xs_sb = rpool.tile([128, NB * LP], DDT, tag="xs")
                    s["xs"] = xs_sb
                    for j in range(NB):
                        nc.vector.memset(xs_sb[:, j * LP:j * LP + 3], 0.0)
                xs_sb = s["xs"]
                t0 = tch * 512
                xt_sb = wpool.tile([128, 8 * 512], DDT, tag="xt", bufs=1)
                for mt in range(8):
                    nc.scalar.dma_start(
                        out=xt_sb[:, mt * 512:(mt + 1) * 512],
                        in_=xT_d.ap()[b, mt * 128:(mt + 1) * 128, t0:t0 + 512],
                    )
                for j in range(NB):
                    xs_ps = ppool.tile([128, 512], F32, tag="mm")
                    for mt in range(8):
                        nc.tensor.matmul(
                            out=xs_ps[:, :],
                            lhsT=w_xs_sb[:, mt * DS + j * 128:
                                         mt * DS + (j + 1) * 128],
                            rhs=xt_sb[:, mt * 512:(mt + 1) * 512],
                            start=(mt == 0),
                            stop=(mt == 7),
                        )
                    nc.scalar.activation(
                        out=xs_sb[:, j * LP + 3 + t0:j * LP + 3 + t0 + 512],
                        in_=xs_ps[:, :],
                        func=ACT.Copy,
                    )

            def convxp_unit(b, tch):
                """depthwise conv (diag matmuls) + silu -> u, then the
                x_proj partial matmul for this time chunk."""
                s = st[b]
                if tch == 0:
                    u_sb = rpool.tile([128, NB * L], DDT, tag="u")
                    s["u"] = u_sb
                xp_st = wpool.tile([97, 512], DDT, tag="xp", bufs=1)
                u_sb, xs_sb = s["u"], s["xs"]
                t0 = tch * 512
                for j in range(NB):
                    xc_ps = ppool.tile([128, 512], F32, tag="mm")
                    for k in range(K_CONV):
                        nc.tensor.matmul(
                            out=xc_ps[:, :],
                            lhsT=conv_diag_sb[:, (j * K_CONV + k) * 128:
                                              (j * K_CONV + k + 1) * 128],
                            rhs=xs_sb[:, j * LP + t0 + k:j * LP + t0 + k + 512],
                            start=(k == 0),
                            stop=(k == K_CONV - 1),
                        )
                    nc.scalar.activation(
                        out=u_sb[:, j * L + t0:j * L + t0 + 512],
                        in_=xc_ps[:, :],
                        func=ACT.Silu,
                        bias=conv_b_sb[:, j:j + 1],
                        scale=1.0,
                    )
                xp_ps = ppool.tile([128, 512], F32, tag="mm")
                for j in range(NB):
                    nc.tensor.matmul(
                        out=xp_ps[0:96, :],
                        lhsT=w_xp_sb[:, j * 96:(j + 1) * 96],
                        rhs=u_sb[:, j * L + t0:j * L + t0 + 512],
                        start=(j == 0),
                        stop=(j == NB - 1),
                    )
                nc.scalar.activation(
                    out=xp_st[0:96, :],
                    in_=xp_ps[0:96, :],
                    func=ACT.Copy,
                )
                nc.scalar.dma_start(
                    out=xdbl_loc[b].ap()[:, t0:t0 + 512],
                    in_=xp_st[0:96, :],
                )

            def ar_unit(b):
                """group AllReduce of x_proj partials (bf16)."""
                nc.gpsimd.collective_compute(
                    "AllReduce",
                    mybir.AluOpType.add,
                    replica_groups=GROUPS,
                    ins=[xdbl_loc[b].ap()[:, :].opt()],
                    outs=[xdbl_red[b].ap()[:, :].opt()],
                )
                s = st[b]
                # dt rows live in the (now dead) xt load tile
                dt_sb = wpool.tile([128, 8 * 512], DDT, tag="xt", bufs=1)
                s["dt"] = dt_sb
                nc.scalar.dma_start(
                    out=dt_sb[0:64, 0:L], in_=xdbl_red[b].ap()[0:64, :]
                )

            def dtexp_unit(b, j):
                """dt_proj matmuls + softplus-exp for one channel block.
                exp(dt_proj + bias) lands in the (dead) xs tile; the Ln
                pass finishes softplus into delta (bf16)."""
                s = st[b]
                if j == 0:
                    delta_sb = dpool.tile([128, NB * L], DDT, tag="delta")
                    s["delta"] = delta_sb
                dt_sb, xs_sb = s["dt"], s["xs"]
                for tch in range(4):
                    t0 = tch * 512
                    dp_ps = ppool.tile([128, 512], F32, tag="mm")
                    nc.tensor.matmul(
                        out=dp_ps[:, :],
                        lhsT=w_dt_sb[:, j * 128:(j + 1) * 128],
                        rhs=dt_sb[0:64, t0:t0 + 512],
                        start=True,
                        stop=True,
                    )
                    # softplus(v+b) = ln(1 + exp(v+b)); no softplus table.
                    nc.scalar.activation(
                        out=xs_sb[:, j * LP + 3 + t0:j * LP + 3 + t0 + 512],
                        in_=dp_ps[:, :],
                        func=ACT.Exp,
                        bias=dtb_sb[:, j:j + 1],
                        scale=1.0,
                    )

            def ln_unit(b, half):
                """second softplus half: delta = ln(1 + sp)."""
                s = st[b]
                delta_sb, xs_sb = s["delta"], s["xs"]
                for j in (half * 2, half * 2 + 1):
                    for tch in range(4):
                        t0 = tch * 512
                        nc.scalar.activation(
                            out=delta_sb[:, j * L + t0:j * L + t0 + 512],
                            in_=xs_sb[:, j * LP + 3 + t0:j * LP + 3 + t0 + 512],
                            func=ACT.Ln,
                            bias=1.0,
                            scale=1.0,
                        )

            def zrec_unit(b, tch):
                """z-proj recomputed from re-loaded xT; silu applied
                straight from PSUM into zs. DMAs via ScalarE's queue."""
                s = st[b]
                if tch == 0:
                    zs_sb = rpool.tile([128, NB * L], DDT, tag="zs")
                    s["zs"] = zs_sb
                zs_sb = s["zs"]
                t0 = tch * 512
                xt3_sb = wpool.tile([128, 8 * 512], DDT, tag="xt", bufs=1)
                for mt in range(8):
                    nc.scalar.dma_start(
                        out=xt3_sb[:, mt * 512:(mt + 1) * 512],
                        in_=xT_d.ap()[b, mt * 128:(mt + 1) * 128, t0:t0 + 512],
                    )
                for j in range(NB):
                    z_ps = opool.tile([128, 512], F32, tag="omm")
                    for mt in range(8):
                        nc.tensor.matmul(
                            out=z_ps[:, :],
                            lhsT=w_z_sb[:, mt * DS + j * 128:
                                        mt * DS + (j + 1) * 128],
                            rhs=xt3_sb[:, mt * 512:(mt + 1) * 512],
                            start=(mt == 0),
                            stop=(mt == 7),
                        )
                    nc.scalar.activation(
                        out=zs_sb[:, j * L + t0:j * L + t0 + 512],
                        in_=z_ps[:, :],
                        func=ACT.Silu,
                    )

            def wmult_unit(b):
                """w = delta*u on DVE (bf16 2x)."""
                s = st[b]
                w_sb = rpool.tile([128, NB * L], DDT, tag="w")
                s["w"] = w_sb
                for j in range(NB):
                    nc.vector.tensor_tensor(
                        out=w_sb[:, j * L:(j + 1) * L],
                        in0=s["delta"][:, j * L:(j + 1) * L],
                        in1=s["u"][:, j * L:(j + 1) * L],
                        op=ALU.mult,
                    )

            def accinit_unit(b):
                """acc = D*u via ScalarE scale-copy."""
                s = st[b]
                acc_sb = rpool.tile([128, NB * L], ADT, tag="acc")
                s["acc"] = acc_sb
                for j in range(NB):
                    nc.scalar.activation(
                        out=acc_sb[:, j * L:(j + 1) * L],
                        in_=s["u"][:, j * L:(j + 1) * L],
                        func=ACT.Copy,
                        scale=dskip_sb[:, j:j + 1],
                    )

            # ---------------- scan loop ----------------

            def stage_emit(b, k):
                """produce inputs for scan iteration k = n*NB + j."""
                s = st[b]
                n, j = divmod(k, NB)
                if j == 0:
                    brep = s3pool.tile([128, L], DDT, tag="brep", bufs=1)
                    nc.sync.dma_start(
                        out=brep[:, :],
                        in_=xdbl_red[b].ap()[R + n:R + n + 1, :]
                        .partition_broadcast(128),
                    )
                    crep = s3pool.tile([128, L], DDT, tag="crep", bufs=1)
                    nc.sync.dma_start(
                        out=crep[:, :],
                        in_=xdbl_red[b].ap()[R + N + n:R + N + n + 1, :]
                        .partition_broadcast(128),
                    )
                    s[("brep", n)] = brep
                    s[("crep", n)] = crep
                dA = s3pool.tile([128, L], SDT, tag="dA", bufs=2)
                nc.scalar.activation(
                    out=dA[:, :],
                    in_=s["delta"][:, j * L:(j + 1) * L],
                    func=ACT.Exp,
                    scale=a_sb[:, j * N + n:j * N + n + 1],
                )
                dBu = s3pool.tile([128, L], SDT, tag="dBu", bufs=1)
                eng = nc.gpsimd if (gp_mod > 0 and k % gp_mod == 0) else nc.vector
                eng.tensor_tensor(
                    out=dBu[:, :],
                    in0=s[("brep", n)][:, :],
                    in1=s["w"][:, j * L:(j + 1) * L],
                    op=ALU.mult,
                )
                if j == NB - 1:
                    del s[("brep", n)]
                s[("dA", k)] = dA
                s[("dBu", k)] = dBu

            def scan_op(b, k):
                s = st[b]
                dA = s.pop(("dA", k))
                dBu = s.pop(("dBu", k))
                h = s3pool.tile([128, L], SDT, tag="h", bufs=2)
                s[("h", k)] = h
                nc.vector.tensor_tensor_scan(
                    out=h[:, :],
                    data0=dA[:, :],
                    data1=dBu[:, :],
                    initial=0.0,
                    op0=ALU.mult,
                    op1=ALU.add,
                )

            def pacc_op(b, k):
                s = st[b]
                n, j = divmod(k, NB)
                h = s.pop(("h", k))
                # p = crep * h (in-place DVE ops run ~4x slow, and buffer
                # reuse across already-emitted writers serializes, so p
                # gets its own rotation)
                p = s3pool.tile([128, L], SDT, tag="p", bufs=2)
                nc.vector.tensor_tensor(
                    out=p[:, :],
                    in0=s[("crep", n)][:, :],
                    in1=h[:, :],
                    op=ALU.mult,
                )
                if j == NB - 1:
                    del s[("crep", n)]
                nc.gpsimd.tensor_tensor(
                    out=s["acc"][:, j * L:(j + 1) * L],
                    in0=s["acc"][:, j * L:(j + 1) * L],
                    in1=p[:, :],
                    op=ALU.add,
                )

            def gates_unit(b):
                """yg = acc * zs (zs already silu'd); into the dead xs
                tile (LP-strided blocks)."""
                s = st[b]
                yg_sb = s["xs"]
                s["yg"] = yg_sb
                for j in range(NB):
                    nc.vector.tensor_tensor(
                        out=yg_sb[:, j * LP + 3:j * LP + 3 + L],
                        in0=s["acc"][:, j * L:(j + 1) * L],
                        in1=s["zs"][:, j * L:(j + 1) * L],
                        op=ALU.mult,
                    )

            def outmm_unit(b, quarter):
                """folded out_proj+fusion matmul, partials to dram."""
                s = st[b]
                yg_sb = s["yg"]
                for tb in range(quarter * 4, quarter * 4 + 4):
                    for eh in range(2):
                        o_ps = opool.tile([128, 512], F32, tag="omm")
                        for j in range(NB):
                            nc.tensor.matmul(
                                out=o_ps[:, :],
                                lhsT=yg_sb[:, j * LP + 3 + tb * 128:
                                           j * LP + 3 + (tb + 1) * 128],
                                rhs=w_out_sb[:, j * DM + eh * 512:
                                             j * DM + (eh + 1) * 512],
                                start=(j == 0),
                                stop=(j == NB - 1),
                            )
                        o_sb = wpool.tile([128, 512], DDT, tag="ot", bufs=1)
                        nc.scalar.activation(
                            out=o_sb[:, :], in_=o_ps[:, :], func=ACT.Copy
                        )
                        nc.scalar.dma_start(
                            out=part_out[b].ap()[tb * 128:(tb + 1) * 128,
                                                 eh * 512:(eh + 1) * 512],
                            in_=o_sb[:, :],
                        )

            def rs_unit(b, half):
                """ReduceScatter one L/2 half: rows [h*1024, (h+1)*1024)
                reduce-scatter to 256 rows per core at
                rs_out[b*512 + h*256 : ...]."""
                HL = L // 2
                q = HL // 4
                nc.gpsimd.collective_compute(
                    "ReduceScatter",
                    mybir.AluOpType.add,
                    replica_groups=GROUPS,
                    ins=[part_out[b].ap()[half * HL:(half + 1) * HL, :].opt()],
                    outs=[rs_out.ap()[b * (L // 4) + half * q:
                                      b * (L // 4) + (half + 1) * q, :].opt()],
                )

            # ---------------- schedule ----------------

            # b0 startup, straight-line (Scalar table order:
            # Copy, Silu, Copy, Exp, Ln, Silu, then scan Exps).
            for tch in range(4):
                mm_unit(0, tch)
            for tch in range(4):
                convxp_unit(0, tch)
            ar_unit(0)
            for j in range(NB):
                dtexp_unit(0, j)
            for half in range(2):
                ln_unit(0, half)
            wmult_unit(0)
            accinit_unit(0)

            # zrec(0) + b1 prep units interleaved into scan(0)'s emission.
            def b1_units():
                for tch in range(4):
                    yield lambda t=tch: zrec_unit(0, t)
                for tch in range(4):
                    yield lambda t=tch: mm_unit(1, t)
                for tch in range(4):
                    yield lambda t=tch: convxp_unit(1, t)
                yield lambda: ar_unit(1)
                for j in range(NB):
                    yield lambda jj=j: dtexp_unit(1, jj)
                for half in range(2):
                    yield lambda h=half: ln_unit(1, h)

            units1 = b1_units()

            def next_unit():
                u = next(units1, None)
                if u:
                    u()

            def scan_loop(b, milestones):
                stage_emit(b, 0)
                for k in range(NI):
                    scan_op(b, k)
                    if k + 1 < NI:
                        stage_emit(b, k + 1)
                    pacc_op(b, k)
                    ms = milestones.get(k)
                    if ms:
                        ms()

            ms0 = {k: next_unit for k in range(2, 2 + 2 * 19, 2)}
            scan_loop(0, ms0)

            wmult_unit(1)
            gates_unit(0)
            accinit_unit(1)

            # outmm(0) / zrec(1) / RS(0) interleaved into scan(1) so the
            # engine FIFOs aren't blocked at the batch boundary.
            ms1 = {
                2: lambda: outmm_unit(0, 0),
                4: lambda: outmm_unit(0, 1),
                6: lambda: rs_unit(0, 0),
                8: lambda: outmm_unit(0, 2),
                10: lambda: outmm_unit(0, 3),
                12: lambda: rs_unit(0, 1),
                14: lambda: zrec_unit(1, 0),
                16: lambda: zrec_unit(1, 1),
                18: lambda: zrec_unit(1, 2),
                20: lambda: zrec_unit(1, 3),
            }
            scan_loop(1, ms1)

            gates_unit(1)
            outmm_unit(1, 0)
            outmm_unit(1, 1)
            rs_unit(1, 0)
            outmm_unit(1, 2)
            outmm_unit(1, 3)
            rs_unit(1, 1)

            nc.sync.dma_start(out=out_d.ap()[:, :], in_=rs_out.ap()[:, :])

    nc.finalize()
    return nc


def _np_dt(name):
    if name == "bfloat16":
        import ml_dtypes
        return np.dtype(ml_dtypes.bfloat16)
    return np.dtype(np.float32)


def _prep_core_inputs(inputs, core, data_dtype="bfloat16"):
    g = core // 4
    j = core % 4
    rows = slice(j * DS, (j + 1) * DS)
    pref = "fwd_" if g == 0 else "bwd_"
    ddt = _np_dt(data_dtype)

    def P(name):
        return np.asarray(inputs[pref + name], dtype=np.float32)

    x = np.asarray(inputs["x"], dtype=np.float32)
    if g == 1:
        x = x[:, ::-1]
    xT = np.ascontiguousarray(x.transpose(0, 2, 1)).astype(ddt)

    in_proj_w = P("in_proj_w")
    w_xs = np.ascontiguousarray(in_proj_w[rows].T).astype(ddt)
    w_z = np.ascontiguousarray(
        in_proj_w[DI + j * DS:DI + (j + 1) * DS].T
    ).astype(ddt)

    conv_w = P("conv_w")[rows, 0, :]          # [512, 4]
    # [128, NB*K_CONV*128]: per (jb, k) a diag(conv_w[jb*128:(jb+1)*128, k])
    conv_diag = np.zeros((128, NB * K_CONV * 128), np.float32)
    for jb in range(NB):
        for k in range(K_CONV):
            blk = slice((jb * K_CONV + k) * 128, (jb * K_CONV + k + 1) * 128)
            conv_diag[:, blk] = np.diag(conv_w[jb * 128:(jb + 1) * 128, k])
    conv_diag = np.ascontiguousarray(conv_diag).astype(ddt)

    conv_b_pack = np.ascontiguousarray(P("conv_b")[rows].reshape(NB, 128).T)
    dtb_pack = np.ascontiguousarray(P("dt_proj_b")[rows].reshape(NB, 128).T)
    dskip_pack = np.ascontiguousarray(P("D")[rows].reshape(NB, 128).T)

    w_xp = np.ascontiguousarray(P("x_proj_w")[:, rows].T).astype(ddt)
    w_dt = np.ascontiguousarray(P("dt_proj_w")[rows].T).astype(ddt)

    A = -np.exp(P("A_log")[rows])             # [512, 16]
    a_pack = np.ascontiguousarray(
        A.reshape(NB, 128, N).transpose(1, 0, 2).reshape(128, NB * N)
    )

    fusion_w = np.asarray(inputs["fusion_w"], dtype=np.float32)
    w_out = np.ascontiguousarray(
        P("out_proj_w")[:, rows].T @ fusion_w[:, g * DM:(g + 1) * DM].T
    ).astype(ddt)

    return {
        "xT": xT,
        "w_xs": w_xs,
        "w_z": w_z,
        "w_xp": w_xp,
        "w_dt": w_dt,
        "w_out": w_out,
        "conv_diag": conv_diag,
        "conv_b": conv_b_pack,
        "dtb": dtb_pack,
        "dskip": dskip_pack,
        "a_pack": a_pack,
    }


LAST_EXEC_NS = None


def kernel(**inputs):
    global LAST_EXEC_NS
    from concourse.bass_utils import run_bass_kernel_spmd

    data_dtype = os.environ.get("KERNEL_DATA_DT", "bfloat16")
    scan_dtype = os.environ.get("KERNEL_SCAN_DT", "bfloat16")
    acc_dtype = os.environ.get("KERNEL_ACC_DT", "float32")
    gp_mod = int(os.environ.get("KERNEL_GP_MOD", "0"))
    scan_inplace = bool(int(os.environ.get("KERNEL_SCAN_INPLACE", "1")))
    key = (data_dtype, scan_dtype, acc_dtype, gp_mod, scan_inplace)
    if key not in _CACHE:
        _CACHE[key] = build_program(
            data_dtype, scan_dtype, acc_dtype, gp_mod, scan_inplace
        )
    nc = _CACHE[key]

    in_maps = [_prep_core_inputs(inputs, c, data_dtype) for c in range(8)]
    trace = bool(int(os.environ.get("KERNEL_TRACE", "0")))
    res = run_bass_kernel_spmd(nc, in_maps, core_ids=list(range(8)), trace=trace)
    LAST_EXEC_NS = res.exec_time_ns

    shards = [np.asarray(res.results[c]["out"], dtype=np.float32)
              for c in range(8)]
    # halved per-b ReduceScatter: core with group-rank j holds, for each
    # (b, half), rows [h*1024 + j*256 : h*1024 + (j+1)*256] of that b's
    # [L, DM] block, stored at rs_out[b*512 + h*256 : ...].

    def assemble(group):
        out = np.empty((B, L, DM), np.float32)
        for b in range(B):
            for h in range(2):
                for j in range(4):
                    rows = shards[group * 4 + j][
                        b * 512 + h * 256:b * 512 + (h + 1) * 256
                    ]
                    out[b, h * 1024 + j * 256:h * 1024 + (j + 1) * 256] = rows
        return out

    fwd = assemble(0)
    bwd = assemble(1)[:, ::-1]
    fusion_b = np.asarray(inputs["fusion_b"], dtype=np.float32)
    return (fwd + bwd + fusion_b).astype(np.float32)
